# revision 1
# baseline (speedup 1.0000x reference)
"""LSTMCell (B=16384, I=H=512) on 8 Trainium2 NeuronCores.

Strategy: data-parallel over the batch (2048 rows/core). Each core computes
gatesT = W @ [x;h]T in transposed layout (gate dim on partitions, batch on the
free dim) so that:
  - the contraction dim (I+H) lands on SBUF partitions for both matmul
    operands with zero on-chip transposes (inputs are pre-transposed on the
    host while sharding),
  - the gate bias is a per-partition vector, applied for free by the ScalarE
    activation instruction,
  - fp32 data runs through the PE at bf16 rate via the float32r dtype
    (moving free dim 512 >= 256).
The stacked gate dim is permuted on the host so each 128-row h-block's four
gate tiles (i, f, g, o) are contiguous in the weight matrix, letting weights
stream in [128, 512] chunks in exactly the order the PE consumes them.
Elementwise LSTM tail (sigmoid/tanh/mul/add) runs on ScalarE + VectorE
overlapped with the matmuls; outputs are stored transposed and un-transposed
on the host.
"""

import numpy as np
from contextlib import ExitStack

_B, _I, _H = 16384, 512, 512
_NC = 8
_BL = _B // _NC          # 2048 batch rows per core
_G = 4 * _H              # 2048 stacked gate dim
_K = _I + _H             # 1024 contraction dim
_BCH = 512               # batch chunk (PSUM bank free size)
_NB = _BL // _BCH        # 4 batch chunks
_NJ = _H // 128          # 4 h-blocks of 128
_NK = _K // 128          # 8 k-chunks of 128
_NT = 4                  # gates (i, f, g, o)

_cache = {}


def _build(reps=1):
    from concourse import bacc
    import concourse.mybir as mybir
    import concourse.tile as tile

    f32 = mybir.dt.float32
    f32r = mybir.dt.float32r
    AF = mybir.ActivationFunctionType

    nc = bacc.Bacc("TRN2", target_bir_lowering=False, debug=False,
                   num_devices=_NC)
    xT = nc.declare_dram_parameter("xT", [_I, _BL], f32r, isOutput=False)
    hT = nc.declare_dram_parameter("hT", [_H, _BL], f32r, isOutput=False)
    cT = nc.declare_dram_parameter("cT", [_H, _BL], f32, isOutput=False)
    # gate dim pre-permuted on host: column block j*512..j*512+512 holds the
    # (i, f, g, o) tiles for h-block j, each 128 wide.
    wT = nc.declare_dram_parameter("wT", [_K, _G], f32r, isOutput=False)
    b2 = nc.declare_dram_parameter("b2", [128, _G // 128], f32, isOutput=False)
    hoT = nc.declare_dram_parameter("hoT", [_H, _BL], f32, isOutput=True)
    coT = nc.declare_dram_parameter("coT", [_H, _BL], f32, isOutput=True)

    with ExitStack() as ctx:
        tc = ctx.enter_context(tile.TileContext(nc))
        wp = ctx.enter_context(tc.tile_pool(name="w", bufs=1))
        xp = ctx.enter_context(tc.tile_pool(name="xh", bufs=1))
        bp = ctx.enter_context(tc.tile_pool(name="bias", bufs=1))
        cp = ctx.enter_context(tc.tile_pool(name="cin", bufs=3))
        ap = ctx.enter_context(tc.tile_pool(name="act", bufs=2))
        op = ctx.enter_context(tc.tile_pool(name="out", bufs=2))
        pp = ctx.enter_context(tc.tile_pool(name="ps", bufs=2, space="PSUM"))

        def body(_iv=None):
            bias_sb = bp.tile([128, _G // 128], f32, tag="bias")
            nc.sync.dma_start(out=bias_sb[:], in_=b2[:])

            # Weight tiles [128k, 512g] per (k, j); activation tiles
            # [128k, 512b] per (k, bc). Issued in the order the PE consumes
            # them: everything group (bc=0, j=0) needs first, then j-blocks,
            # then remaining batch chunks.
            w_sb = [[None] * _NJ for _ in range(_NK)]
            xh_sb = [[None] * _NB for _ in range(_NK)]

            def load_w(k, j):
                t_ = wp.tile([128, _NT * 128], f32r, tag=f"w{k}_{j}")
                nc.sync.dma_start(
                    out=t_[:], in_=wT[k * 128:(k + 1) * 128,
                                      j * 512:(j + 1) * 512])
                w_sb[k][j] = t_

            def load_xh(k, bc):
                t_ = xp.tile([128, _BCH], f32r, tag=f"xh{k}_{bc}")
                src = xT if k < _NK // 2 else hT
                r = (k % (_NK // 2)) * 128
                nc.sync.dma_start(
                    out=t_[:], in_=src[r:r + 128,
                                       bc * _BCH:(bc + 1) * _BCH])
                xh_sb[k][bc] = t_

            for k in range(_NK):
                load_w(k, 0)
                load_xh(k, 0)
            for j in range(1, _NJ):
                for k in range(_NK):
                    load_w(k, j)
            for bc in range(1, _NB):
                for k in range(_NK):
                    load_xh(k, bc)

            for bc in range(_NB):
                bsl = slice(bc * _BCH, (bc + 1) * _BCH)
                for j in range(_NJ):
                    ps = []
                    for t in range(_NT):
                        pstile = pp.tile([128, _BCH], f32, tag=f"ps{t}")
                        for k in range(_NK):
                            nc.tensor.matmul(
                                pstile[:],
                                w_sb[k][j][:, t * 128:(t + 1) * 128],
                                xh_sb[k][bc][:],
                                start=(k == 0), stop=(k == _NK - 1),
                            )
                        ps.append(pstile)
                    c_sb = cp.tile([128, _BCH], f32, tag="c")
                    nc.scalar.dma_start(out=c_sb[:],
                                        in_=cT[j * 128:(j + 1) * 128, bsl])
                    gI = ap.tile([128, _BCH], f32, tag="gI")
                    gF = ap.tile([128, _BCH], f32, tag="gF")
                    gG = ap.tile([128, _BCH], f32, tag="gG")
                    gO = ap.tile([128, _BCH], f32, tag="gO")
                    bcol = j * _NT
                    nc.scalar.activation(gI[:], ps[0][:], AF.Sigmoid,
                                         bias=bias_sb[:, bcol + 0:bcol + 1])
                    nc.scalar.activation(gF[:], ps[1][:], AF.Sigmoid,
                                         bias=bias_sb[:, bcol + 1:bcol + 2])
                    nc.scalar.activation(gG[:], ps[2][:], AF.Tanh,
                                         bias=bias_sb[:, bcol + 2:bcol + 3])
                    nc.scalar.activation(gO[:], ps[3][:], AF.Sigmoid,
                                         bias=bias_sb[:, bcol + 3:bcol + 4])
                    newc = op.tile([128, _BCH], f32, tag="newc")
                    newh = op.tile([128, _BCH], f32, tag="newh")
                    nc.vector.tensor_mul(gF[:], gF[:], c_sb[:])   # f * c
                    nc.vector.tensor_mul(gI[:], gI[:], gG[:])     # i * g
                    nc.vector.tensor_add(newc[:], gF[:], gI[:])
                    nc.scalar.activation(gG[:], newc[:], AF.Tanh)
                    nc.vector.tensor_mul(newh[:], gO[:], gG[:])
                    nc.scalar.dma_start(out=coT[j * 128:(j + 1) * 128, bsl],
                                        in_=newc[:])
                    nc.scalar.dma_start(out=hoT[j * 128:(j + 1) * 128, bsl],
                                        in_=newh[:])

        if reps == 1:
            body()
        else:
            with tc.For_i(0, reps, 1):
                body()
    nc.compile()
    return nc


# Gate-dim permutation: position j*4 + t  <-  original gate tile t*4 + j
# (tile index into the stacked-gates dim of 16 x 128 rows).
def _gate_perm():
    perm = np.empty(_G, np.int64)
    pos = 0
    for j in range(_NJ):
        for t in range(_NT):
            src = (t * _NJ + j) * 128
            perm[pos:pos + 128] = np.arange(src, src + 128)
            pos += 128
    return perm


def _host_shards(x, h, c, Wi, bi, Wh, bh):
    perm = _gate_perm()
    W = np.concatenate([np.asarray(Wi, np.float32),
                        np.asarray(Wh, np.float32)], axis=1)    # [G, K]
    wTv = np.ascontiguousarray(W[perm].T)                        # [K, G] permuted
    b = (np.asarray(bi, np.float32) + np.asarray(bh, np.float32))[perm]
    b2 = np.ascontiguousarray(b.reshape(_G // 128, 128).T)       # [128, G/128]
    in_maps = []
    for s in range(_NC):
        sl = slice(s * _BL, (s + 1) * _BL)
        in_maps.append({
            "xT": np.ascontiguousarray(np.asarray(x, np.float32)[sl].T),
            "hT": np.ascontiguousarray(np.asarray(h, np.float32)[sl].T),
            "cT": np.ascontiguousarray(np.asarray(c, np.float32)[sl].T),
            "wT": wTv,
            "b2": b2,
        })
    return in_maps


def kernel(x, h, c, Wi, bi, Wh, bh):
    from concourse.bass_utils import run_bass_kernel_spmd

    nc = _cache.get("nc")
    if nc is None:
        nc = _build()
        _cache["nc"] = nc

    in_maps = _host_shards(x, h, c, Wi, bi, Wh, bh)
    res = run_bass_kernel_spmd(nc, in_maps, list(range(_NC)))

    h_out = np.empty((_B, _H), np.float32)
    c_out = np.empty((_B, _H), np.float32)
    for s in range(_NC):
        sl = slice(s * _BL, (s + 1) * _BL)
        h_out[sl] = res.results[s]["hoT"].T
        c_out[sl] = res.results[s]["coT"].T
    return h_out, c_out



# revision 4
# speedup vs baseline: 1.3542x; 1.3542x over previous
"""LSTMCell (B=16384, I=H=512) on 8 Trainium2 NeuronCores.

Strategy: data-parallel over the batch (2048 rows/core). Each core computes
gatesT = W @ [x;h]T in transposed layout (gate dim on partitions, batch on the
free dim) so that:
  - the contraction dim (I+H) lands on SBUF partitions for both matmul
    operands with zero on-chip transposes (inputs are pre-transposed on the
    host while sharding),
  - the gate bias is a per-partition vector, applied for free by the ScalarE
    activation instruction.
All tensors are bf16 on the wire (inputs, weights, outputs): halves HBM
traffic vs f32 and keeps the PE at 1 row/cycle with fast (FWL) weight loads
that hide under the previous matmul's stream. Moving operand is [128, 1024]
(bf16 max), halving instruction count vs 512-wide chunks; each PSUM tile
spans 2 banks, 4 gate tiles fill all 8 banks, and bank reuse across (j, ch)
groups gives the matmul/activation overlap.
The stacked gate dim is permuted on the host so each 128-row h-block's four
gate tiles (i, f, g, o) are contiguous in the weight matrix.
Elementwise LSTM tail (sigmoid/tanh/mul/add) runs on ScalarE + VectorE
overlapped with the matmuls; outputs are stored transposed in bf16 and
un-transposed/upcast on the host.
"""

import numpy as np
import ml_dtypes
from contextlib import ExitStack

_B, _I, _H = 16384, 512, 512
_NC = 8
_BL = _B // _NC          # 2048 batch rows per core
_G = 4 * _H              # 2048 stacked gate dim
_K = _I + _H             # 1024 contraction dim
_BCH = 512               # batch chunk (PSUM bank free size)
_NB = _BL // _BCH        # 4 batch chunks
_NJ = _H // 128          # 4 h-blocks of 128
_NK = _K // 128          # 8 k-chunks of 128
_NT = 4                  # gates (i, f, g, o)

_cache = {}


def _build(reps=1):
    from concourse import bacc
    import concourse.mybir as mybir
    import concourse.tile as tile

    f32 = mybir.dt.float32
    bf16 = mybir.dt.bfloat16
    AF = mybir.ActivationFunctionType

    nc = bacc.Bacc("TRN2", target_bir_lowering=False, debug=False,
                   num_devices=_NC)
    xT = nc.declare_dram_parameter("xT", [_I, _BL], bf16, isOutput=False)
    hT = nc.declare_dram_parameter("hT", [_H, _BL], bf16, isOutput=False)
    cT = nc.declare_dram_parameter("cT", [_H, _BL], bf16, isOutput=False)
    # gate dim pre-permuted on host: column block j*512..j*512+512 holds the
    # (i, f, g, o) tiles for h-block j, each 128 wide.
    wT = nc.declare_dram_parameter("wT", [_K, _G], bf16, isOutput=False)
    b2 = nc.declare_dram_parameter("b2", [128, _G // 128], f32, isOutput=False)
    hoT = nc.declare_dram_parameter("hoT", [_H, _BL], bf16, isOutput=True)
    coT = nc.declare_dram_parameter("coT", [_H, _BL], bf16, isOutput=True)

    with ExitStack() as ctx:
        tc = ctx.enter_context(tile.TileContext(nc))
        wp = ctx.enter_context(tc.tile_pool(name="w", bufs=1))
        xp = ctx.enter_context(tc.tile_pool(name="xh", bufs=2))
        bp = ctx.enter_context(tc.tile_pool(name="bias", bufs=1))
        cp = ctx.enter_context(tc.tile_pool(name="cin", bufs=3))
        ap = ctx.enter_context(tc.tile_pool(name="act", bufs=2))
        op = ctx.enter_context(tc.tile_pool(name="out", bufs=2))
        pp = ctx.enter_context(tc.tile_pool(name="ps", bufs=2, space="PSUM"))

        def body(_iv=None):
            bias_sb = bp.tile([128, _G // 128], f32, tag="bias")
            nc.sync.dma_start(out=bias_sb[:], in_=b2[:])

            # Weight tiles [128k, 512g] per (k, j); activation tiles
            # [128k, 1024b] per (k, ch). Issued in the order the PE consumes
            # them: the (ch=0, j=0) group's tiles first (interleaved so the
            # PE can start after the first pair), then remaining j-blocks'
            # weights, then the second batch chunk.
            w_sb = [[None] * _NJ for _ in range(_NK)]
            xh_sb = [[None] * _NB for _ in range(_NK)]

            def load_w(k, j):
                t_ = wp.tile([128, _NT * 128], bf16, tag=f"w{k}_{j}")
                nc.sync.dma_start(
                    out=t_[:], in_=wT[k * 128:(k + 1) * 128,
                                      j * 512:(j + 1) * 512])
                w_sb[k][j] = t_

            def load_xh(k, ch):
                t_ = xp.tile([128, _BCH], bf16, tag=f"xh{k}_{ch}")
                src = xT if k < _NK // 2 else hT
                r = (k % (_NK // 2)) * 128
                nc.sync.dma_start(
                    out=t_[:], in_=src[r:r + 128,
                                       ch * _BCH:(ch + 1) * _BCH])
                xh_sb[k][ch] = t_

            for k in range(_NK):
                load_w(k, 0)
                load_xh(k, 0)
            for j in range(1, _NJ):
                for k in range(_NK):
                    load_w(k, j)
            for ch in range(1, _NB):
                for k in range(_NK):
                    load_xh(k, ch)

            for ch in range(_NB):
                bsl = slice(ch * _BCH, (ch + 1) * _BCH)
                for j in range(_NJ):
                    c_sb = cp.tile([128, _BCH], bf16, tag="c")
                    nc.scalar.dma_start(out=c_sb[:],
                                        in_=cT[j * 128:(j + 1) * 128, bsl])
                    ps = []
                    for t in range(_NT):
                        pstile = pp.tile([128, _BCH], f32, tag=f"ps{t}")
                        for k in range(_NK):
                            nc.tensor.matmul(
                                pstile[:],
                                w_sb[k][j][:, t * 128:(t + 1) * 128],
                                xh_sb[k][ch][:],
                                start=(k == 0), stop=(k == _NK - 1),
                            )
                        ps.append(pstile)
                    gI = ap.tile([128, _BCH], f32, tag="gI")
                    gF = ap.tile([128, _BCH], f32, tag="gF")
                    gG = ap.tile([128, _BCH], f32, tag="gG")
                    gO = ap.tile([128, _BCH], f32, tag="gO")
                    bcol = j * _NT
                    nc.scalar.activation(gI[:], ps[0][:], AF.Sigmoid,
                                         bias=bias_sb[:, bcol + 0:bcol + 1])
                    nc.scalar.activation(gF[:], ps[1][:], AF.Sigmoid,
                                         bias=bias_sb[:, bcol + 1:bcol + 2])
                    nc.scalar.activation(gG[:], ps[2][:], AF.Tanh,
                                         bias=bias_sb[:, bcol + 2:bcol + 3])
                    nc.scalar.activation(gO[:], ps[3][:], AF.Sigmoid,
                                         bias=bias_sb[:, bcol + 3:bcol + 4])
                    newc = op.tile([128, _BCH], bf16, tag="newc")
                    newh = op.tile([128, _BCH], bf16, tag="newh")
                    nc.vector.tensor_mul(gF[:], gF[:], c_sb[:])   # f * c
                    nc.vector.tensor_mul(gI[:], gI[:], gG[:])     # i * g
                    nc.vector.tensor_add(newc[:], gF[:], gI[:])
                    nc.scalar.activation(gG[:], newc[:], AF.Tanh)
                    nc.vector.tensor_mul(newh[:], gO[:], gG[:])
                    nc.scalar.dma_start(out=coT[j * 128:(j + 1) * 128, bsl],
                                        in_=newc[:])
                    nc.scalar.dma_start(out=hoT[j * 128:(j + 1) * 128, bsl],
                                        in_=newh[:])

        if reps == 1:
            body()
        else:
            with tc.For_i(0, reps, 1):
                body()
    nc.compile()
    return nc


# Gate-dim permutation: position j*4 + t  <-  original gate tile t*4 + j
# (tile index into the stacked-gates dim of 16 x 128 rows).
def _gate_perm():
    perm = np.empty(_G, np.int64)
    pos = 0
    for j in range(_NJ):
        for t in range(_NT):
            src = (t * _NJ + j) * 128
            perm[pos:pos + 128] = np.arange(src, src + 128)
            pos += 128
    return perm


_BF16 = ml_dtypes.bfloat16


def _host_shards(x, h, c, Wi, bi, Wh, bh):
    perm = _gate_perm()
    W = np.concatenate([np.asarray(Wi, np.float32),
                        np.asarray(Wh, np.float32)], axis=1)    # [G, K]
    wTv = np.ascontiguousarray(W[perm].T).astype(_BF16)          # [K, G] permuted
    b = (np.asarray(bi, np.float32) + np.asarray(bh, np.float32))[perm]
    b2 = np.ascontiguousarray(b.reshape(_G // 128, 128).T)       # [128, G/128]
    xTf = np.asarray(x, np.float32).T
    hTf = np.asarray(h, np.float32).T
    cTf = np.asarray(c, np.float32).T
    in_maps = []
    for s in range(_NC):
        sl = slice(s * _BL, (s + 1) * _BL)
        in_maps.append({
            "xT": np.ascontiguousarray(xTf[:, sl]).astype(_BF16),
            "hT": np.ascontiguousarray(hTf[:, sl]).astype(_BF16),
            "cT": np.ascontiguousarray(cTf[:, sl]).astype(_BF16),
            "wT": wTv,
            "b2": b2,
        })
    return in_maps


def kernel(x, h, c, Wi, bi, Wh, bh):
    from concourse.bass_utils import run_bass_kernel_spmd

    nc = _cache.get("nc")
    if nc is None:
        nc = _build()
        _cache["nc"] = nc

    in_maps = _host_shards(x, h, c, Wi, bi, Wh, bh)
    res = run_bass_kernel_spmd(nc, in_maps, list(range(_NC)))

    h_out = np.empty((_B, _H), np.float32)
    c_out = np.empty((_B, _H), np.float32)
    for s in range(_NC):
        sl = slice(s * _BL, (s + 1) * _BL)
        h_out[sl] = res.results[s]["hoT"].T.astype(np.float32)
        c_out[sl] = res.results[s]["coT"].T.astype(np.float32)
    return h_out, c_out
